# revision 1
# baseline (speedup 1.0000x reference)
"""Trainium2 Bass kernel for nn_Affinity (graph-matching affinity matrix).

Math per sample (validated against the reference):
  out[(a,c),(b,c')] = sum_{e2,e1} G2[a,e2] H2[b,e2] Me[e2,e1] G1[c,e1] H1[c',e1]
                      + diag(vec(Mp))

Key structural fact: G1/H1 columns are one-hot (edge e1 has exactly one
head c(e1) and tail c'(e1), and the (c,c') pairs are distinct across
edges), so for every (a,b) block the 32x32 (c,c') submatrix has exactly
E=96 nonzeros:
  out[(a,c(e1)), (b,c'(e1))] = Z[e1, (a,b)],
  Z[e1,(a,b)] = sum_e2 Me[e2,e1] G2[a,e2] H2[b,e2].
The dense 1024x1024 output is a pure placement of the 96x1024 tensor Z
(plus the diagonal of vec(Mp)).

Device (1 sample per NeuronCore, fully static instruction stream):
  1. Row-major edge ranks via masked prefix-scan (both graphs fused);
     one-hot rank expansion; G and H recovered TOGETHER by 8 accumulating
     matmuls against constant [head-select | tail-select] matrices
     (head row p//4 and tail col 8*(p%4)+k are static per flat slot).
  2. Edge affinity Me via fp16 matmuls (lam prep on PE: psum(l + l^T),
     relu on the copy out).
  3. Z = Me-contraction with P2[e2,(a,b)] = G2T[e2,a]*H2T[e2,b].
  4. Edge head/tail indices via one-hot @ partition-iota matmuls.
  5. Packed fp16 DMA out: [Z | c | c' | MpT]. All numerics run on
     device; the host unshard only places device-computed values at
     device-computed indices (the Kronecker one-hot scatter) and casts.
"""

import numpy as np

import concourse.bacc as bacc
import concourse.bass as bass
import concourse.mybir as mybir
import concourse.tile as tile
from concourse.bass_utils import run_bass_kernel_spmd

F32 = mybir.dt.float32
F16 = mybir.dt.float16
I32 = mybir.dt.int32
ALU = mybir.AluOpType
AX = mybir.AxisListType
AF = mybir.ActivationFunctionType

B, N, D, E = 8, 32, 128, 96
NCORES = 8

# zout column layout: [z0 | z1 | idx | mpt | z2 | z3]
ZC = [0, 256, 546, 802]
OUTW = 1058


def make_selgh():
    """selgh[:, 64k:64k+64] = [head-select | tail-select(k)] for flat slot
    (p, k): head row = p//4, tail col = 8*(p%4)+k."""
    p = np.arange(128)
    sel = np.zeros((128, 512), np.float16)
    for k in range(8):
        sel[:, 64 * k:64 * k + 32] = (p[:, None] // 4 == np.arange(32)[None, :])
        sel[:, 64 * k + 32:64 * k + 64] = (
            8 * (p[:, None] % 4) + k == np.arange(32)[None, :])
    return sel


def build_program(debug: bool = False):
    nc = bacc.Bacc("TRN2", target_bir_lowering=False, debug=debug,
                   num_devices=NCORES)
    ain = nc.dram_tensor("ain", [128, 16], F32, kind="ExternalInput")
    selgh = nc.dram_tensor("selgh", [128, 512], F16, kind="ExternalInput")
    lufb = nc.dram_tensor("lufb", [128, 576], F16, kind="ExternalInput")
    out = nc.dram_tensor("out", [96, OUTW], F16, kind="ExternalOutput")

    with tile.TileContext(nc) as tc:
        with tc.tile_pool(name="sb", bufs=1) as sb, \
             tc.tile_pool(name="ps", bufs=1, space="PSUM") as ps:
            # ---- input DMAs: A first (critical, SP), selgh on the Act
            # queue, lam/U/F second on SP ----
            asb = sb.tile([128, 16], F32, tag="asb")
            nc.sync.dma_start(out=asb[:], in_=ain[:, :])
            selsb = sb.tile([128, 512], F16, tag="selsb")
            nc.scalar.dma_start(out=selsb[:], in_=selgh[:, :])
            lusb = sb.tile([128, 576], F16, tag="lusb")
            nc.sync.dma_start(out=lusb[:], in_=lufb[:, :])
            l1_16, l2_16 = lusb[:, 0:128], lusb[:, 128:256]
            u1_16, u2_16 = lusb[:, 256:288], lusb[:, 288:320]
            f1t16, f2t16 = lusb[0:32, 320:448], lusb[0:32, 448:576]

            # ---- on-device constants (run during the input-DMA window) ----
            it32 = sb.tile([128, 129], I32, tag="it32")
            nc.gpsimd.iota(it32[:], pattern=[[1, 129]], base=0,
                           channel_multiplier=0)
            pi32 = sb.tile([128, 1], I32, tag="pi32")
            nc.gpsimd.iota(pi32[:], pattern=[[1, 1]], base=0,
                           channel_multiplier=1)
            io16 = sb.tile([128, 129], F16, tag="io16")
            nc.vector.tensor_copy(out=io16[:], in_=it32[:])
            pi16 = sb.tile([128, 1], F16, tag="pi16")
            nc.vector.tensor_copy(out=pi16[:], in_=pi32[:])
            io32 = sb.tile([128, 128], F32, tag="io32")
            nc.vector.tensor_copy(out=io32[:], in_=it32[:, 0:128])
            pf32 = sb.tile([128, 1], F32, tag="pf32")
            nc.vector.tensor_copy(out=pf32[:], in_=pi32[:])
            id16 = sb.tile([128, 128], F16, tag="id16")
            nc.vector.tensor_scalar(out=id16[:], in0=io16[:, 0:128],
                                    scalar1=pf32[:, 0:1], scalar2=None,
                                    op0=ALU.is_equal)
            su32 = sb.tile([128, 128], F32, tag="su32")
            nc.vector.tensor_scalar(out=su32[:], in0=io32[:],
                                    scalar1=pf32[:, 0:1], scalar2=None,
                                    op0=ALU.is_gt)
            zout = sb.tile([96, OUTW], F16, tag="zout")
            nc.vector.memset(zout[0:96, 514:546], 0.0)

            # ---- edge ranks (row-major) for both graphs ----
            maskb = sb.tile([128, 16], F32, tag="maskb")
            nc.vector.tensor_scalar(out=maskb[:], in0=asb[:], scalar1=0.49,
                                    scalar2=None, op0=ALU.is_ge)
            m3 = maskb[:].rearrange("p (g k) -> p g k", k=8)
            s2 = sb.tile([128, 2], F32, tag="s2")
            nc.vector.tensor_reduce(out=s2[:], in_=m3, axis=AX.X, op=ALU.add)
            pbase = ps.tile([128, 2], F32, tag="psA", bufs=1)
            nc.tensor.matmul(out=pbase[:], lhsT=su32[:], rhs=s2[:],
                             start=True, stop=True)
            r0 = sb.tile([128, 16], F32, tag="r0")
            r1h = sb.tile([128, 16], F32, tag="r1h")
            for g in (1, 0):
                nc.vector.tensor_tensor_scan(
                    out=r0[:, 8 * g:8 * (g + 1)],
                    data0=maskb[:, 8 * g:8 * (g + 1)],
                    data1=maskb[:, 8 * g:8 * (g + 1)],
                    initial=pbase[:, g:g + 1],
                    op0=ALU.add, op1=ALU.bypass)
                nc.vector.tensor_tensor(out=r1h[:, 8 * g:8 * (g + 1)],
                                        in0=r0[:, 8 * g:8 * (g + 1)],
                                        in1=maskb[:, 8 * g:8 * (g + 1)],
                                        op=ALU.mult)

            # ---- one-hot rank expansion + per-graph [G|H] matmuls ----
            oh = sb.tile([128, 1536], F16, tag="oh")
            ghps = {}
            ghb = {}
            for gi, g in enumerate((1, 0)):
                base = 768 * gi
                for k in range(8):
                    nc.vector.tensor_scalar(
                        out=oh[:, base + 96 * k:base + 96 * (k + 1)],
                        in0=io16[:, 1:97],
                        scalar1=r1h[:, 8 * g + k:8 * g + k + 1], scalar2=None,
                        op0=ALU.is_equal)
                if g == 1:
                    pg_ = ps.tile([32, 96], F32, tag="psG2", bufs=1)
                    ph_ = ps.tile([32, 96], F32, tag="psH2", bufs=1)
                    pgv, phv = pg_[:], ph_[:]
                else:
                    pg_ = ps.tile([32, 192], F32, tag="psGH0", bufs=1)
                    pgv, phv = pg_[:, 0:96], pg_[:, 96:192]
                for k in range(8):
                    nc.tensor.matmul(
                        out=pgv, lhsT=selsb[:, 64 * k:64 * k + 32],
                        rhs=oh[:, base + 96 * k:base + 96 * (k + 1)],
                        start=(k == 0), stop=(k == 7))
                b_ = sb.tile([32, 192], F16, tag=f"ghb{g}")
                if g == 1:
                    nc.scalar.copy(out=b_[:, 0:96], in_=pgv)
                for k in range(8):
                    nc.tensor.matmul(
                        out=phv,
                        lhsT=selsb[:, 64 * k + 32:64 * (k + 1)],
                        rhs=oh[:, base + 96 * k:base + 96 * (k + 1)],
                        start=(k == 0), stop=(k == 7))
                if g == 1:
                    nc.scalar.copy(out=b_[:, 96:192], in_=phv)
                ghps[g] = pg_
                ghb[g] = b_
            g2sb, h2sb = ghb[1][:, 0:96], ghb[1][:, 96:192]

            # transposes of g2/h2 (regular matmuls vs identity) + copy
            trps = ps.tile([96, 64], F32, tag="psA", bufs=1)
            nc.tensor.matmul(out=trps[:, 0:32], lhsT=g2sb,
                             rhs=id16[0:32, 0:32], start=True, stop=True)
            nc.tensor.matmul(out=trps[:, 32:64], lhsT=h2sb,
                             rhs=id16[0:32, 0:32], start=True, stop=True)
            ght16 = sb.tile([96, 64], F16, tag="ght16")
            nc.scalar.copy(out=ght16[:], in_=trps[:])

            # Y feats: yg|yh in one psum, one Act copy
            yyps = ps.tile([128, 192], F32, tag="psF", bufs=1)
            nc.tensor.matmul(out=yyps[:, 0:96], lhsT=f2t16, rhs=g2sb,
                             start=True, stop=True)
            nc.tensor.matmul(out=yyps[:, 96:192], lhsT=f2t16, rhs=h2sb,
                             start=True, stop=True)
            yy16 = sb.tile([128, 192], F16, tag="yy16")
            nc.scalar.copy(out=yy16[:], in_=yyps[:])

            # graph-1 [G;H] single copy
            nc.scalar.copy(out=ghb[0][:], in_=ghps[0][:])
            g1sb, h1sb = ghb[0][:, 0:96], ghb[0][:, 96:192]

            # lam prep: psum_i = l_i + l_i^T (PE), relu on DVE copy-out.
            # wait-ts keeps the scheduler from hoisting these above the
            # incidence matmuls (their DMA lands late).
            lp16 = []
            with tc.tile_wait_until(0.0046):
                for i, l_ in enumerate((l1_16, l2_16)):
                    lp_ = ps.tile([128, 128], F32, tag="psD", bufs=3)
                    nc.tensor.matmul(out=lp_[:], lhsT=id16[:], rhs=l_,
                                     start=True, stop=False)
                    nc.tensor.matmul(out=lp_[:], lhsT=l_, rhs=id16[:],
                                     start=False, stop=True)
                    l16_ = sb.tile([128, 128], F16, tag=f"lp16_{i}")
                    nc.vector.tensor_scalar(out=l16_[:], in0=lp_[:],
                                            scalar1=0.0, scalar2=None,
                                            op0=ALU.max)
                    lp16.append(l16_)

            # P2[e2,(a,b)] = G2T[e2,a]*H2T[e2,b] on DVE (2 halves)
            p2 = sb.tile([96, 1024], F16, tag="p2")
            g2b, h2b = bass.broadcast_tensor_aps(
                ght16[:, 0:32].unsqueeze(2), ght16[:, 32:64].unsqueeze(1))
            for ph in range(2):
                nc.vector.tensor_tensor(
                    out=p2[:, 512 * ph:512 * (ph + 1)].rearrange(
                        "p (a b) -> p a b", b=32),
                    in0=g2b[:, 16 * ph:16 * (ph + 1), :],
                    in1=h2b[:, 16 * ph:16 * (ph + 1), :], op=ALU.mult)

            # T0 = l1p@YG + l2p@YH ; T1 = l2p@YG + l1p@YH  (one psum)
            ttps = ps.tile([128, 192], F32, tag="psGH0", bufs=1)
            for i, (la, lb) in enumerate(((lp16[0], lp16[1]),
                                          (lp16[1], lp16[0]))):
                nc.tensor.matmul(out=ttps[:, 96 * i:96 * (i + 1)],
                                 lhsT=la[:], rhs=yy16[:, 0:96],
                                 start=True, stop=False)
                nc.tensor.matmul(out=ttps[:, 96 * i:96 * (i + 1)],
                                 lhsT=lb[:], rhs=yy16[:, 96:192],
                                 start=False, stop=True)
            tt16 = sb.tile([128, 192], F16, tag="tt16")
            nc.scalar.copy(out=tt16[:], in_=ttps[:])

            # X feats (copy on DVE after p2)
            xxps = ps.tile([128, 192], F32, tag="psF", bufs=1)
            nc.tensor.matmul(out=xxps[:, 0:96], lhsT=f1t16, rhs=g1sb,
                             start=True, stop=True)
            nc.tensor.matmul(out=xxps[:, 96:192], lhsT=f1t16, rhs=h1sb,
                             start=True, stop=True)
            xx16 = sb.tile([128, 192], F16, tag="xx16")
            nc.vector.tensor_copy(out=xx16[:], in_=xxps[:])

            # Me = XG^T T0 + XH^T T1  (96, 96), copy on DVE
            meps = ps.tile([96, 96], F32, tag="psH2", bufs=1)
            nc.tensor.matmul(out=meps[:], lhsT=xx16[:, 0:96],
                             rhs=tt16[:, 0:96], start=True, stop=False)
            nc.tensor.matmul(out=meps[:], lhsT=xx16[:, 96:192],
                             rhs=tt16[:, 96:192], start=False, stop=True)
            me16 = sb.tile([96, 96], F16, tag="me16")
            nc.vector.tensor_copy(out=me16[:], in_=meps[:])

            # edge head/tail indices of graph 1 + MpT (gate only DMA-B)
            eps_ = ps.tile([96, 2], F32, tag="psG2", bufs=1)
            nc.tensor.matmul(out=eps_[:, 0:1], lhsT=g1sb, rhs=pi16[0:32, :],
                             start=True, stop=True)
            nc.tensor.matmul(out=eps_[:, 1:2], lhsT=h1sb, rhs=pi16[0:32, :],
                             start=True, stop=True)
            mpt = ps.tile([32, 32], F32, tag="psG2", bufs=1)
            nc.tensor.matmul(out=mpt[:], lhsT=u2_16, rhs=u1_16,
                             start=True, stop=True)

            # Z = Me^T-contraction @ P2, 4 chunks of 256; copies alternate
            # DVE/Act; DMA-A (z0|z1) chases the first half, DMA-B the rest.
            for k in range(4):
                zps = ps.tile([96, 256], F32, tag="psD", bufs=3)
                nc.tensor.matmul(out=zps[:], lhsT=me16[:],
                                 rhs=p2[:, 256 * k:256 * (k + 1)],
                                 start=True, stop=True)
                eng = nc.vector.tensor_copy if k % 2 == 0 else (
                    lambda out, in_: nc.scalar.copy(out=out, in_=in_))
                eng(out=zout[:, ZC[k]:ZC[k] + 256], in_=zps[:])
                if k == 1:
                    nc.sync.dma_start(out=out[:, 0:512], in_=zout[:, 0:512])
            nc.vector.tensor_copy(out=zout[:, 512:514], in_=eps_[:])
            nc.vector.tensor_copy(out=zout[0:32, 514:546], in_=mpt[:])
            nc.sync.dma_start(out=out[:, 512:OUTW], in_=zout[:, 512:OUTW])
    nc.compile()
    return nc


def make_in_maps(inputs: dict) -> list:
    inputs = {k: np.asarray(v, dtype=np.float32) for k, v in inputs.items()}
    sel = make_selgh()
    in_maps = []
    for b in range(B):
        ain = np.concatenate([
            inputs["A_src"][b].reshape(128, 8),
            inputs["A_tgt"][b].reshape(128, 8),
        ], axis=1).astype(np.float32)
        lufb = np.zeros((128, 576), np.float16)
        lufb[:, 0:128] = inputs["lambda1"]
        lufb[:, 128:256] = inputs["lambda2"]
        lufb[:, 256:288] = inputs["U_src"][b]
        lufb[:, 288:320] = inputs["U_tgt"][b]
        lufb[0:32, 320:448] = inputs["F_src"][b].T
        lufb[0:32, 448:576] = inputs["F_tgt"][b].T
        in_maps.append({
            "ain": np.ascontiguousarray(ain),
            "selgh": sel,
            "lufb": np.ascontiguousarray(lufb),
        })
    return in_maps


_NC_CACHE = {}


def _assemble(packed: np.ndarray) -> np.ndarray:
    """Place device-computed Z values at device-computed (c, c') indices.

    out[(a,c(e)), (b,c'(e))] = Z[e,(a,b)]; out[i,i] += vec(Mp)[i].
    Pure placement + fp16->fp32 cast; no arithmetic on input data.
    """
    z = np.concatenate([packed[:, 0:512], packed[:, 546:1058]],
                       axis=1).astype(np.float32).reshape(E, 32, 32)
    c = np.rint(packed[:, 512].astype(np.float32)).astype(np.int64)
    cp = np.rint(packed[:, 513].astype(np.float32)).astype(np.int64)
    mpt = packed[0:32, 514:546].astype(np.float32)       # MpT[c, a]
    outm = np.zeros((1024, 1024), np.float32)
    o4 = outm.reshape(32, 32, 32, 32)
    o4[:, c, :, cp] = z                                  # axes (e, a, b)
    outm[np.arange(1024), np.arange(1024)] += mpt.T.ravel()
    return outm


def kernel(trace: bool = False, **inputs) -> np.ndarray:
    if "nc" not in _NC_CACHE:
        _NC_CACHE["nc"] = build_program()
    nc = _NC_CACHE["nc"]
    in_maps = make_in_maps(inputs)
    res = run_bass_kernel_spmd(nc, in_maps, core_ids=list(range(NCORES)),
                               trace=trace)
    _NC_CACHE["last_results"] = res
    outs = [_assemble(res.results[b]["out"]) for b in range(B)]
    return np.stack(outs).astype(np.float32)

